# revision 13
# baseline (speedup 1.0000x reference)
"""MethylSPWNet forward pass on 8 Trainium2 NeuronCores.

Heavy part: WX[b, p] = sum_{i: idx[i]==p} x[b, i] * w[i]  (segment reduce,
x is [64, 800000] f32, idx sorted, 128 segments of ~6250).

Strategy (feature-parallel: core c owns the 16 uniform-grid feature
blocks [16c, 16c+16), i.e. columns [100000c, 100000(c+1)) of x, for all
64 batch rows):
  - Per 8-row batch group, the core's 100000 features are viewed as
    [128 partitions, 6250]: partition p = (batch row r = p//16, block
    b = p%16). One DMA per group, 25 KB contiguous per partition.
  - Since idx is sorted and segments are near-uniform, segment 16c+b
    lives almost entirely in block 16c+b, spilling only a few hundred
    features into the edges of the neighbor blocks. Host folds w into
    three masks: wmA (segment == own block, [16, 6250] per core -- DMA'd
    once and broadcast to all 128 partitions on-chip, SBUF->SBUF, no HBM
    traffic), wmB (segment == block-1, leading columns only), wmC
    (segment == block+1, trailing columns only).
  - Device: per group, three fused DVE scalar_tensor_tensor ops
    (elementwise multiply + free-axis reduce in one pass) -> [128, 1]
    accumulators = WX^T entries for 8 rows x 16 segments.
  - Host combines the block-shifted spill accumulators into WX, then
    runs the tiny BN/CancelOut/MLP/softmax tail ([64,128]) in numpy.

HBM traffic per core is x's 25.6 MB + ~0.5 MB of masks. The last group
is DMA'd and reduced in three pieces to shorten the kernel tail.
"""

import sys

import numpy as np

if "/opt/trn_rl_repo" not in sys.path:
    sys.path.insert(0, "/opt/trn_rl_repo")

from contextlib import ExitStack

import concourse.bass as bass
import concourse.mybir as mybir
from concourse.bass_utils import run_bass_kernel_spmd

B, N, P = 64, 800000, 128
G = N // P  # 6250 features per uniform grid block
NCORES = 8
BPC = P // NCORES  # 16 segment blocks per core
NG = B // 8  # 8 batch groups of 8 rows
FPC = N // NCORES  # 100000 features per core
# Last group is DMA'd and reduced in three pieces so the final DVE op
# after the last byte lands is small: [0:H1), [H1:H2), [H2:G)
H1 = 3125
H2 = 5461
EPS = 1e-5

TRACE = False  # test harness sets True to collect an NTFF profile
LAST_RESULT = None  # BassKernelResults of the most recent device run

_nc_cache: dict = {}


BUFS = 4  # x-group buffer slots


def _build_bass(SB: int, SC: int) -> bass.Bass:
    """Raw bass (manual semaphores): SP issues DMAs, DVE does one fused
    multiply+reduce per mask per 8-row group. Tile's auto-generated
    kernel-tail drain trips a walrus 'too many sync waits' limit in this
    container, so the pipeline is hand-synced.

    Semaphore discipline: a wait for the total of a *closed set* of DMAs
    is safe (all must finish to reach the total); a partial count over
    concurrent DMAs is not, because the 16 per-SDMA-engine increments of
    different DMAs interleave. Hence one sem per x-slot use.
    """
    nc = bass.Bass()
    f32 = mybir.dt.float32
    LAST = NG - 1
    # xs[g, r, b, :] = x[8g+r, core_off + b*G : core_off + (b+1)*G]
    xs = nc.dram_tensor("xs", [NG, 8, BPC, G], f32, kind="ExternalInput")
    wa = nc.dram_tensor("wa", [BPC, G], f32, kind="ExternalInput")
    wb = nc.dram_tensor("wb", [P, SB], f32, kind="ExternalInput")
    wc = nc.dram_tensor("wc", [P, SC], f32, kind="ExternalInput")
    # out cols: [0:NG+2) = A (last group split in three pieces),
    # [NG+2 : 2*NG+2) = B, [2*NG+2 : 3*NG+2) = C
    out = nc.dram_tensor("out", [P, 3 * NG + 2], f32, kind="ExternalOutput")

    mult = mybir.AluOpType.mult

    with ExitStack() as ctx:
        ctx.enter_context(nc.Block())
        wa_t = ctx.enter_context(nc.sbuf_tensor("wa_t", [P, G], f32))
        wb_t = ctx.enter_context(nc.sbuf_tensor("wb_t", [P, SB], f32))
        wc_t = ctx.enter_context(nc.sbuf_tensor("wc_t", [P, SC], f32))
        accall = ctx.enter_context(nc.sbuf_tensor("accall", [P, 3 * NG + 2], f32))
        accA = accall.ap()[:, 0 : NG + 2]
        accB = accall.ap()[:, NG + 2 : 2 * NG + 2]
        accC = accall.ap()[:, 2 * NG + 2 : 3 * NG + 2]
        scrA = ctx.enter_context(nc.sbuf_tensor("scrA", [P, G], f32))
        scrB = ctx.enter_context(nc.sbuf_tensor("scrB", [P, SB], f32))
        scrC = ctx.enter_context(nc.sbuf_tensor("scrC", [P, SC], f32))
        xbuf = [
            ctx.enter_context(nc.sbuf_tensor(f"xb{i}", [P, G], f32))
            for i in range(BUFS)
        ]
        s_m = ctx.enter_context(nc.semaphore("s_m"))  # wb+wc (closed pair)
        s_wa = ctx.enter_context(nc.semaphore("s_wa"))  # wa [0:16] load
        s_wab = ctx.enter_context(nc.semaphore("s_wab"))  # wa broadcast x7
        s_x = [ctx.enter_context(nc.semaphore(f"s_x{i}")) for i in range(BUFS)]
        s_l1 = ctx.enter_context(nc.semaphore("s_l1"))  # last group piece 2
        s_l2 = ctx.enter_context(nc.semaphore("s_l2"))  # last group piece 3
        s_out = ctx.enter_context(nc.semaphore("s_out"))
        s_cmp = ctx.enter_context(nc.semaphore("s_cmp"))  # DVE groups done

        # --- SP stream: input DMAs (serial FIFO on the SP HWDGE ring) ---
        nc.sync.dma_start(out=wb_t.ap(), in_=wb[:]).then_inc(s_m, 16)
        nc.sync.dma_start(out=wc_t.ap(), in_=wc[:]).then_inc(s_m, 16)
        nc.sync.dma_start(out=wa_t.ap()[0:BPC], in_=wa[:]).then_inc(s_wa, 16)
        for g in range(NG):
            if g >= BUFS:  # WAR: slot reused once its group is consumed
                nc.sync.wait_ge(s_cmp, g - BUFS + 1)
            slot = xbuf[g % BUFS].ap()
            if g == LAST:  # split the last group so DVE overlaps its DMA
                nc.sync.dma_start(out=slot[:, 0:H1], in_=xs[g][:, :, 0:H1]).then_inc(
                    s_x[g % BUFS], 16
                )
                nc.sync.dma_start(out=slot[:, H1:H2], in_=xs[g][:, :, H1:H2]).then_inc(
                    s_l1, 16
                )
                nc.sync.dma_start(out=slot[:, H2:G], in_=xs[g][:, :, H2:G]).then_inc(
                    s_l2, 16
                )
            else:
                nc.sync.dma_start(out=slot, in_=xs[g]).then_inc(s_x[g % BUFS], 16)
        nc.sync.wait_ge(s_cmp, NG)
        nc.sync.dma_start(out=out[:], in_=accall.ap()).then_inc(s_out, 16)
        nc.sync.wait_ge(s_out, 16)

        # --- ACT stream: broadcast wa from partitions [0:16) to the other
        # seven 16-partition groups (SBUF->SBUF, off the HBM path). ---
        nc.scalar.wait_ge(s_wa, 16)
        for k in range(1, 8):
            nc.scalar.dma_start(
                out=wa_t.ap()[k * BPC : (k + 1) * BPC], in_=wa_t.ap()[0:BPC]
            ).then_inc(s_wab, 16)

        # --- DVE stream: fused multiply+reduce per mask per group. The
        # race detector needs an explicit completion wait before scratch
        # reuse (write-ack is async); one s_cmp wait per group covers all
        # three scratches via same-engine program order. ---
        nc.vector.wait_ge(s_m, 32)
        nc.vector.wait_ge(s_wab, 112)
        for g in range(NG):
            nc.vector.wait_ge(s_x[g % BUFS], 16 * (g // BUFS + 1))
            xt = xbuf[g % BUFS].ap()
            if g > 0:
                nc.vector.wait_ge(s_cmp, g)  # scrA/B/C free (group g-1 done)
            if g == LAST:
                nc.vector.scalar_tensor_tensor(
                    out=scrA.ap()[:, 0:H1], in0=xt[:, 0:H1], scalar=1.0,
                    in1=wa_t.ap()[:, 0:H1], op0=mult, op1=mult,
                    accum_out=accA[:, g : g + 1],
                )
                nc.vector.scalar_tensor_tensor(
                    out=scrB.ap(), in0=xt[:, 0:SB], scalar=1.0, in1=wb_t.ap(),
                    op0=mult, op1=mult,
                    accum_out=accB[:, g : g + 1],
                )
                nc.vector.wait_ge(s_l1, 16)
                nc.vector.scalar_tensor_tensor(
                    out=scrA.ap()[:, H1:H2], in0=xt[:, H1:H2], scalar=1.0,
                    in1=wa_t.ap()[:, H1:H2], op0=mult, op1=mult,
                    accum_out=accA[:, g + 1 : g + 2],
                )
                nc.vector.wait_ge(s_l2, 16)
                nc.vector.scalar_tensor_tensor(
                    out=scrA.ap()[:, H2:G], in0=xt[:, H2:G], scalar=1.0,
                    in1=wa_t.ap()[:, H2:G], op0=mult, op1=mult,
                    accum_out=accA[:, g + 2 : g + 3],
                )
                nc.vector.scalar_tensor_tensor(
                    out=scrC.ap(), in0=xt[:, G - SC : G], scalar=1.0,
                    in1=wc_t.ap(), op0=mult, op1=mult,
                    accum_out=accC[:, g : g + 1],
                ).then_inc(s_cmp, 1)
            else:
                nc.vector.scalar_tensor_tensor(
                    out=scrA.ap(), in0=xt, scalar=1.0, in1=wa_t.ap(),
                    op0=mult, op1=mult,
                    accum_out=accA[:, g : g + 1],
                )
                nc.vector.scalar_tensor_tensor(
                    out=scrB.ap(), in0=xt[:, 0:SB], scalar=1.0, in1=wb_t.ap(),
                    op0=mult, op1=mult,
                    accum_out=accB[:, g : g + 1],
                )
                nc.vector.scalar_tensor_tensor(
                    out=scrC.ap(), in0=xt[:, G - SC : G], scalar=1.0,
                    in1=wc_t.ap(), op0=mult, op1=mult,
                    accum_out=accC[:, g : g + 1],
                ).then_inc(s_cmp, 1)
    return nc


def _prep_masks(w: np.ndarray, idx: np.ndarray):
    """Fold w into block-aligned masks (global [P, ...] layout; sliced /
    tiled per core by the caller). Returns None if idx strays more than
    one block from uniform (never happens for near-uniform sorted idx;
    caller falls back to a CPU scatter)."""
    blk = np.arange(N, dtype=np.int64) // G
    d = np.asarray(idx, np.int64) - blk
    if not bool(np.all(np.abs(d) <= 1)):
        return None
    col = np.arange(N, dtype=np.int64) % G
    wmA = np.where(d == 0, w, 0.0).astype(np.float32).reshape(P, G)

    selB = d == -1
    SB = int(col[selB].max()) + 1 if selB.any() else 1
    SB = min(G, max(16, -(-SB // 16) * 16))
    wmB = np.zeros((P, SB), np.float32)
    wmB[blk[selB], col[selB]] = w[selB]

    selC = d == 1
    SC = G - int(col[selC].min()) if selC.any() else 1
    SC = min(G, max(16, -(-SC // 16) * 16))
    wmC = np.zeros((P, SC), np.float32)
    wmC[blk[selC], col[selC] - (G - SC)] = w[selC]
    return wmA, wmB, wmC, SB, SC


def _core_in_map(x, wmA, wmB, wmC, c):
    xs = np.ascontiguousarray(
        x[:, c * FPC : (c + 1) * FPC].reshape(NG, 8, BPC, G)
    )
    return {
        "xs": xs,
        "wa": np.ascontiguousarray(wmA[c * BPC : (c + 1) * BPC]),
        "wb": np.ascontiguousarray(np.tile(wmB[c * BPC : (c + 1) * BPC], (8, 1))),
        "wc": np.ascontiguousarray(np.tile(wmC[c * BPC : (c + 1) * BPC], (8, 1))),
    }


def _combine_core(o, c, WX):
    """Fold one core's accumulators into WX [B, P]. Partition p = (row
    r = p//16 within group, block b = p%16); column g = batch group."""
    a = o[:, : NG + 2]
    bacc = o[:, NG + 2 : 2 * NG + 2]
    cacc = o[:, 2 * NG + 2 : 3 * NG + 2]
    segs = c * BPC + np.arange(BPC)

    A = np.array(a[:, :NG])
    A[:, NG - 1] += a[:, NG] + a[:, NG + 1]  # last group's three pieces
    # [p, g] -> rows 8g+r, segment segs[b]
    A = A.reshape(8, BPC, NG)  # [r, b, g]
    WX[:, segs] += A.transpose(2, 0, 1).reshape(B, BPC)

    Bm = bacc.reshape(8, BPC, NG).transpose(2, 0, 1).reshape(B, BPC)
    # block b's leading spill belongs to segment segs[b]-1
    tgt = segs - 1
    ok = tgt >= 0
    WX[:, tgt[ok]] += Bm[:, ok]

    Cm = cacc.reshape(8, BPC, NG).transpose(2, 0, 1).reshape(B, BPC)
    tgt = segs + 1
    ok = tgt <= P - 1
    WX[:, tgt[ok]] += Cm[:, ok]


def _segment_reduce_device(x: np.ndarray, wmA, wmB, wmC, SB, SC) -> np.ndarray:
    global LAST_RESULT
    key = (SB, SC)
    nc = _nc_cache.get(key)
    if nc is None:
        nc = _build_bass(SB, SC)
        _nc_cache[key] = nc

    in_maps = [_core_in_map(x, wmA, wmB, wmC, c) for c in range(NCORES)]
    LAST_RESULT = run_bass_kernel_spmd(
        nc, in_maps, core_ids=list(range(NCORES)), trace=TRACE
    )
    results = LAST_RESULT.results

    WX = np.zeros((B, P), np.float32)
    for c in range(NCORES):
        _combine_core(results[c]["out"], c, WX)
    return WX


def _segment_reduce_cpu(x: np.ndarray, w: np.ndarray, idx: np.ndarray):
    WX = np.zeros((B, P), np.float32)
    np.add.at(WX.T, np.asarray(idx, np.int64), (x * w).T)
    return WX


def _bn(z, g, b):
    m = z.mean(axis=0)
    v = np.mean((z - m) ** 2, axis=0)
    return (z - m) / np.sqrt(v + EPS) * g + b


def kernel(**inputs) -> tuple:
    x = np.ascontiguousarray(np.asarray(inputs["x"], np.float32))
    w = np.asarray(inputs["w"], np.float32)
    idx = np.asarray(inputs["idx"])

    masks = _prep_masks(w, idx)
    if masks is not None:
        WX = _segment_reduce_device(x, *masks)
    else:
        WX = _segment_reduce_cpu(x, w, idx)

    # Tiny MLP head on host ([64, 128] scale).
    f = np.float32
    Z = _bn(
        np.maximum(WX, 0),
        np.asarray(inputs["bn0_g"], f),
        np.asarray(inputs["bn0_b"], f),
    )
    Z = Z * (1.0 / (1.0 + np.exp(-np.asarray(inputs["co_w"], f))))
    h = _bn(
        np.maximum(Z @ np.asarray(inputs["W1"], f) + np.asarray(inputs["b1"], f), 0),
        np.asarray(inputs["bn1_g"], f),
        np.asarray(inputs["bn1_b"], f),
    )
    h = _bn(
        np.maximum(h @ np.asarray(inputs["W2"], f) + np.asarray(inputs["b2"], f), 0),
        np.asarray(inputs["bn2_g"], f),
        np.asarray(inputs["bn2_b"], f),
    )
    logits = h @ np.asarray(inputs["Wo"], f) + np.asarray(inputs["bo"], f)
    logits = logits - logits.max(axis=-1, keepdims=True)
    e = np.exp(logits)
    y = e / e.sum(axis=-1, keepdims=True)
    return (y.astype(np.float32), Z.astype(np.float32))


# revision 14
# speedup vs baseline: 1.2179x; 1.2179x over previous
"""MethylSPWNet forward pass on 8 Trainium2 NeuronCores.

Heavy part: WX[b, p] = sum_{i: idx[i]==p} x[b, i] * w[i]  (segment reduce,
x is [64, 800000] f32, idx sorted, 128 segments of ~6250).

Strategy (batch-parallel, 8 rows of x per core):
  - Per batch row, view the 800000 features as [128 partitions, 6250]
    (partition p = contiguous feature block p). Since idx is sorted and
    segments are near-uniform, segment p lives almost entirely in block p,
    spilling only a few hundred features into the edges of blocks p+-1.
  - Host folds w into three masks: wmA (features whose segment == own
    block), wmB (segment == block-1, nonzero only in leading columns),
    wmC (segment == block+1, nonzero only in trailing columns).
  - Device: one DMA per row ([128, 6250], fully contiguous per partition)
    plus three fused DVE scalar_tensor_tensor ops (elementwise multiply +
    free-axis reduce in a single pass) -> per-row accumulators [128, 1].
  - Host combines the partition-shifted accumulators into WX, then runs
    the tiny BN/CancelOut/MLP/softmax tail ([64,128] scale) in numpy.

DMA traffic is x's 25.6 MB per core + 3.7 MB of masks -> memory roofline.
The last row is split in pieces so the tail DVE op overlaps its own DMA.
"""

import sys

import numpy as np

if "/opt/trn_rl_repo" not in sys.path:
    sys.path.insert(0, "/opt/trn_rl_repo")

from contextlib import ExitStack

import concourse.bass as bass
import concourse.mybir as mybir
from concourse.bass_utils import run_bass_kernel_spmd

B, N, P = 64, 800000, 128
G = N // P  # 6250 features per partition block
# Last row is DMA'd and reduced in three pieces so the final DVE op after
# the last byte lands is small: [0:H1), [H1:H2), [H2:G)
H1 = 3125
H2 = 5461
NCORES = 8
RPC = B // NCORES  # batch rows per core
EPS = 1e-5

TRACE = False  # test harness sets True to collect an NTFF profile
LAST_RESULT = None  # BassKernelResults of the most recent device run

_nc_cache: dict = {}


BUFS = 4  # x-row buffer slots


def _build_bass(SB: int, SC: int) -> bass.Bass:
    """Raw bass (manual semaphores): SP issues DMAs, DVE does one fused
    multiply+reduce per mask per row. Tile's auto-generated kernel-tail
    drain trips a walrus 'too many sync waits' limit in this container,
    so the pipeline is hand-synced.

    Semaphore discipline: a wait for the total of a *closed set* of DMAs
    is safe (all must finish to reach the total); a partial count over
    concurrent DMAs is not, because the 16 per-SDMA-engine increments of
    different DMAs interleave. Hence one sem per x-row slot use.
    """
    nc = bass.Bass()
    f32 = mybir.dt.float32
    LAST = RPC - 1
    xs = nc.dram_tensor("xs", [RPC, P, G], f32, kind="ExternalInput")
    wa = nc.dram_tensor("wa", [P, G], f32, kind="ExternalInput")
    wb = nc.dram_tensor("wb", [P, SB], f32, kind="ExternalInput")
    wc = nc.dram_tensor("wc", [P, SC], f32, kind="ExternalInput")
    # out cols: [0:RPC+2) = A (last row split in three pieces),
    # [RPC+2 : 2*RPC+2) = B, [2*RPC+2 : 3*RPC+2) = C
    out = nc.dram_tensor("out", [P, 3 * RPC + 2], f32, kind="ExternalOutput")

    mult = mybir.AluOpType.mult

    with ExitStack() as ctx:
        ctx.enter_context(nc.Block())
        wa_t = ctx.enter_context(nc.sbuf_tensor("wa_t", [P, G], f32))
        wb_t = ctx.enter_context(nc.sbuf_tensor("wb_t", [P, SB], f32))
        wc_t = ctx.enter_context(nc.sbuf_tensor("wc_t", [P, SC], f32))
        accall = ctx.enter_context(nc.sbuf_tensor("accall", [P, 3 * RPC + 2], f32))
        accA = accall.ap()[:, 0 : RPC + 2]
        accB = accall.ap()[:, RPC + 2 : 2 * RPC + 2]
        accC = accall.ap()[:, 2 * RPC + 2 : 3 * RPC + 2]
        scrA = ctx.enter_context(nc.sbuf_tensor("scrA", [P, G], f32))
        scrB = ctx.enter_context(nc.sbuf_tensor("scrB", [P, SB], f32))
        scrC = ctx.enter_context(nc.sbuf_tensor("scrC", [P, SC], f32))
        xbuf = [
            ctx.enter_context(nc.sbuf_tensor(f"xb{i}", [P, G], f32))
            for i in range(BUFS)
        ]
        s_m = ctx.enter_context(nc.semaphore("s_m"))  # wb+wc (closed pair)
        s_wa = ctx.enter_context(nc.semaphore("s_wa"))
        s_x = [ctx.enter_context(nc.semaphore(f"s_x{i}")) for i in range(BUFS)]
        s_l1 = ctx.enter_context(nc.semaphore("s_l1"))  # last row piece 2
        s_l2 = ctx.enter_context(nc.semaphore("s_l2"))  # last row piece 3
        s_out = ctx.enter_context(nc.semaphore("s_out"))
        s_cmp = ctx.enter_context(nc.semaphore("s_cmp"))  # DVE rows done

    # --- SP stream: input DMAs (serial FIFO on the HWDGE ring) ---
        nc.sync.dma_start(out=wb_t.ap(), in_=wb[:]).then_inc(s_m, 16)
        nc.sync.dma_start(out=wc_t.ap(), in_=wc[:]).then_inc(s_m, 16)
        nc.sync.dma_start(out=wa_t.ap(), in_=wa[:]).then_inc(s_wa, 16)
        for r in range(RPC):
            if r >= BUFS:  # WAR: slot reused once its row is consumed
                nc.sync.wait_ge(s_cmp, r - BUFS + 1)
            slot = xbuf[r % BUFS].ap()
            if r == LAST:  # split the last row so DVE overlaps its DMA
                nc.sync.dma_start(out=slot[:, 0:H1], in_=xs[r][:, 0:H1]).then_inc(
                    s_x[r % BUFS], 16
                )
                nc.sync.dma_start(out=slot[:, H1:H2], in_=xs[r][:, H1:H2]).then_inc(
                    s_l1, 16
                )
                nc.sync.dma_start(out=slot[:, H2:G], in_=xs[r][:, H2:G]).then_inc(
                    s_l2, 16
                )
            else:
                nc.sync.dma_start(out=slot, in_=xs[r]).then_inc(s_x[r % BUFS], 16)
        nc.sync.wait_ge(s_cmp, RPC)
        nc.sync.dma_start(out=out[:], in_=accall.ap()).then_inc(s_out, 16)
        nc.sync.wait_ge(s_out, 16)

        # --- DVE stream: fused multiply+reduce per mask per row. The
        # race detector needs an explicit completion wait before scratch
        # reuse (write-ack is async); one s_cmp wait per row covers all
        # three scratches via same-engine program order. ---
        nc.vector.wait_ge(s_m, 32)
        nc.vector.wait_ge(s_wa, 16)
        for r in range(RPC):
            nc.vector.wait_ge(s_x[r % BUFS], 16 * (r // BUFS + 1))
            xt = xbuf[r % BUFS].ap()
            if r > 0:
                nc.vector.wait_ge(s_cmp, r)  # scrA/B/C free (row r-1 done)
            if r == LAST:
                nc.vector.scalar_tensor_tensor(
                    out=scrA.ap()[:, 0:H1], in0=xt[:, 0:H1], scalar=1.0,
                    in1=wa_t.ap()[:, 0:H1], op0=mult, op1=mult,
                    accum_out=accA[:, r : r + 1],
                )
                nc.vector.scalar_tensor_tensor(
                    out=scrB.ap(), in0=xt[:, 0:SB], scalar=1.0, in1=wb_t.ap(),
                    op0=mult, op1=mult,
                    accum_out=accB[:, r : r + 1],
                )
                nc.vector.wait_ge(s_l1, 16)
                nc.vector.scalar_tensor_tensor(
                    out=scrA.ap()[:, H1:H2], in0=xt[:, H1:H2], scalar=1.0,
                    in1=wa_t.ap()[:, H1:H2], op0=mult, op1=mult,
                    accum_out=accA[:, r + 1 : r + 2],
                )
                nc.vector.wait_ge(s_l2, 16)
                nc.vector.scalar_tensor_tensor(
                    out=scrA.ap()[:, H2:G], in0=xt[:, H2:G], scalar=1.0,
                    in1=wa_t.ap()[:, H2:G], op0=mult, op1=mult,
                    accum_out=accA[:, r + 2 : r + 3],
                )
                nc.vector.scalar_tensor_tensor(
                    out=scrC.ap(), in0=xt[:, G - SC : G], scalar=1.0,
                    in1=wc_t.ap(), op0=mult, op1=mult,
                    accum_out=accC[:, r : r + 1],
                ).then_inc(s_cmp, 1)
            else:
                nc.vector.scalar_tensor_tensor(
                    out=scrA.ap(), in0=xt, scalar=1.0, in1=wa_t.ap(),
                    op0=mult, op1=mult,
                    accum_out=accA[:, r : r + 1],
                )
                nc.vector.scalar_tensor_tensor(
                    out=scrB.ap(), in0=xt[:, 0:SB], scalar=1.0, in1=wb_t.ap(),
                    op0=mult, op1=mult,
                    accum_out=accB[:, r : r + 1],
                )
                nc.vector.scalar_tensor_tensor(
                    out=scrC.ap(), in0=xt[:, G - SC : G], scalar=1.0,
                    in1=wc_t.ap(), op0=mult, op1=mult,
                    accum_out=accC[:, r : r + 1],
                ).then_inc(s_cmp, 1)
    return nc


def _prep_masks(w: np.ndarray, idx: np.ndarray):
    """Fold w into block-aligned fp16 masks. Returns None if idx strays
    more than one block from uniform (never happens for near-uniform
    sorted idx; caller falls back to a CPU scatter)."""
    blk = np.arange(N, dtype=np.int64) // G
    d = np.asarray(idx, np.int64) - blk
    if not bool(np.all(np.abs(d) <= 1)):
        return None
    col = np.arange(N, dtype=np.int64) % G
    wmA = np.where(d == 0, w, 0.0).astype(np.float32).reshape(P, G)

    selB = d == -1
    SB = int(col[selB].max()) + 1 if selB.any() else 1
    SB = min(G, max(16, -(-SB // 16) * 16))
    wmB = np.zeros((P, SB), np.float32)
    wmB[blk[selB], col[selB]] = w[selB]

    selC = d == 1
    SC = G - int(col[selC].min()) if selC.any() else 1
    SC = min(G, max(16, -(-SC // 16) * 16))
    wmC = np.zeros((P, SC), np.float32)
    wmC[blk[selC], col[selC] - (G - SC)] = w[selC]
    return wmA, wmB, wmC, SB, SC


def _segment_reduce_device(x: np.ndarray, wmA, wmB, wmC, SB, SC) -> np.ndarray:
    global LAST_RESULT
    key = (SB, SC)
    nc = _nc_cache.get(key)
    if nc is None:
        nc = _build_bass(SB, SC)
        _nc_cache[key] = nc

    xr = x.reshape(B, P, G)
    in_maps = [
        {
            "xs": xr[c * RPC : (c + 1) * RPC],
            "wa": wmA,
            "wb": wmB,
            "wc": wmC,
        }
        for c in range(NCORES)
    ]
    LAST_RESULT = run_bass_kernel_spmd(
        nc, in_maps, core_ids=list(range(NCORES)), trace=TRACE
    )
    results = LAST_RESULT.results

    WXT = np.zeros((P, B), np.float32)
    for c in range(NCORES):
        o = results[c]["out"]
        a = o[:, : RPC + 2]
        b = o[:, RPC + 2 : 2 * RPC + 2]
        cc = o[:, 2 * RPC + 2 :]
        cols = slice(c * RPC, (c + 1) * RPC)
        wxt = np.empty((P, RPC), np.float32)
        wxt[:, : RPC - 1] = a[:, : RPC - 1]
        wxt[:, RPC - 1] = a[:, RPC - 1] + a[:, RPC] + a[:, RPC + 1]  # 3 pieces
        wxt[: P - 1] += b[1:]  # block p's spill belongs to segment p-1
        wxt[1:] += cc[: P - 1]  # block p's spill belongs to segment p+1
        WXT[:, cols] = wxt
    return WXT.T


def _segment_reduce_cpu(x: np.ndarray, w: np.ndarray, idx: np.ndarray):
    WX = np.zeros((B, P), np.float32)
    np.add.at(WX.T, np.asarray(idx, np.int64), (x * w).T)
    return WX


def _bn(z, g, b):
    m = z.mean(axis=0)
    v = np.mean((z - m) ** 2, axis=0)
    return (z - m) / np.sqrt(v + EPS) * g + b


def kernel(**inputs) -> tuple:
    x = np.ascontiguousarray(np.asarray(inputs["x"], np.float32))
    w = np.asarray(inputs["w"], np.float32)
    idx = np.asarray(inputs["idx"])

    masks = _prep_masks(w, idx)
    if masks is not None:
        WX = _segment_reduce_device(x, *masks)
    else:
        WX = _segment_reduce_cpu(x, w, idx)

    # Tiny MLP head on host ([64, 128] scale).
    f = np.float32
    Z = _bn(
        np.maximum(WX, 0),
        np.asarray(inputs["bn0_g"], f),
        np.asarray(inputs["bn0_b"], f),
    )
    Z = Z * (1.0 / (1.0 + np.exp(-np.asarray(inputs["co_w"], f))))
    h = _bn(
        np.maximum(Z @ np.asarray(inputs["W1"], f) + np.asarray(inputs["b1"], f), 0),
        np.asarray(inputs["bn1_g"], f),
        np.asarray(inputs["bn1_b"], f),
    )
    h = _bn(
        np.maximum(h @ np.asarray(inputs["W2"], f) + np.asarray(inputs["b2"], f), 0),
        np.asarray(inputs["bn2_g"], f),
        np.asarray(inputs["bn2_b"], f),
    )
    logits = h @ np.asarray(inputs["Wo"], f) + np.asarray(inputs["bo"], f)
    logits = logits - logits.max(axis=-1, keepdims=True)
    e = np.exp(logits)
    y = e / e.sum(axis=-1, keepdims=True)
    return (y.astype(np.float32), Z.astype(np.float32))


# revision 15
# speedup vs baseline: 1.4017x; 1.1510x over previous
"""MethylSPWNet forward pass on 8 Trainium2 NeuronCores.

Heavy part: WX[b, p] = sum_{i: idx[i]==p} x[b, i] * w[i]  (segment reduce,
x is [64, 800000] f32, idx sorted, 128 segments of ~6250).

Strategy (batch-parallel, 8 rows of x per core):
  - Per batch row, view the 800000 features as [128 partitions, 6250]
    (partition p = contiguous feature block p). Since idx is sorted and
    segments are near-uniform, segment p lives almost entirely in block p,
    spilling only a few hundred features into the edges of blocks p+-1.
  - Host folds w into three masks: wmA (features whose segment == own
    block), wmB (segment == block-1, nonzero only in leading columns),
    wmC (segment == block+1, nonzero only in trailing columns).
  - Device: one DMA per row ([128, 6250], fully contiguous per partition)
    plus three fused DVE scalar_tensor_tensor ops (elementwise multiply +
    free-axis reduce in a single pass) -> per-row accumulators [128, 1].
  - Host combines the partition-shifted accumulators into WX, then runs
    the tiny BN/CancelOut/MLP/softmax tail ([64,128] scale) in numpy.

DMA traffic is x's 25.6 MB per core + 3.7 MB of masks -> memory roofline.
The last row is split in pieces so the tail DVE op overlaps its own DMA.
"""

import sys

import numpy as np

if "/opt/trn_rl_repo" not in sys.path:
    sys.path.insert(0, "/opt/trn_rl_repo")

from contextlib import ExitStack

import concourse.bass as bass
import concourse.mybir as mybir
from concourse.bass_utils import run_bass_kernel_spmd

B, N, P = 64, 800000, 128
G = N // P  # 6250 features per partition block
# Last row is DMA'd and reduced in three pieces so the final DVE op after
# the last byte lands is small: [0:H1), [H1:H2), [H2:G)
H1 = 3125
H2 = 5461
NCORES = 8
RPC = B // NCORES  # batch rows per core
EPS = 1e-5

TRACE = False  # test harness sets True to collect an NTFF profile
LAST_RESULT = None  # BassKernelResults of the most recent device run

_nc_cache: dict = {}


BUFS = 4  # x-row buffer slots


def _build_bass(SB: int, SC: int) -> bass.Bass:
    """Raw bass (manual semaphores): SP issues DMAs, DVE does one fused
    multiply+reduce per mask per row. Tile's auto-generated kernel-tail
    drain trips a walrus 'too many sync waits' limit in this container,
    so the pipeline is hand-synced.

    Semaphore discipline: a wait for the total of a *closed set* of DMAs
    is safe (all must finish to reach the total); a partial count over
    concurrent DMAs is not, because the 16 per-SDMA-engine increments of
    different DMAs interleave. Hence one sem per x-row slot use.
    """
    nc = bass.Bass()
    f32 = mybir.dt.float32
    LAST = RPC - 1
    xs = nc.dram_tensor("xs", [RPC, P, G], f32, kind="ExternalInput")
    wall = nc.dram_tensor("wall", [P, G + SB + SC], f32, kind="ExternalInput")
    # out cols: [0:RPC+2) = A (last row split in three pieces),
    # [RPC+2 : 2*RPC+2) = B, [2*RPC+2 : 3*RPC+2) = C
    out = nc.dram_tensor("out", [P, 3 * RPC + 2], f32, kind="ExternalOutput")

    mult = mybir.AluOpType.mult

    with ExitStack() as ctx:
        ctx.enter_context(nc.Block())
        wall_t = ctx.enter_context(nc.sbuf_tensor("wall_t", [P, G + SB + SC], f32))
        wa_t = wall_t.ap()[:, 0:G]
        wb_t = wall_t.ap()[:, G : G + SB]
        wc_t = wall_t.ap()[:, G + SB : G + SB + SC]
        accall = ctx.enter_context(nc.sbuf_tensor("accall", [P, 3 * RPC + 2], f32))
        accA = accall.ap()[:, 0 : RPC + 2]
        accB = accall.ap()[:, RPC + 2 : 2 * RPC + 2]
        accC = accall.ap()[:, 2 * RPC + 2 : 3 * RPC + 2]
        scrA = ctx.enter_context(nc.sbuf_tensor("scrA", [P, G], f32))
        scrB = ctx.enter_context(nc.sbuf_tensor("scrB", [P, SB], f32))
        scrC = ctx.enter_context(nc.sbuf_tensor("scrC", [P, SC], f32))
        xbuf = [
            ctx.enter_context(nc.sbuf_tensor(f"xb{i}", [P, G], f32))
            for i in range(BUFS)
        ]
        s_m = ctx.enter_context(nc.semaphore("s_m"))  # packed masks + out
        s_x = [ctx.enter_context(nc.semaphore(f"s_x{i}")) for i in range(BUFS)]
        s_l1 = ctx.enter_context(nc.semaphore("s_l1"))  # last row piece 2
        s_l2 = ctx.enter_context(nc.semaphore("s_l2"))  # last row piece 3
        s_cmp = ctx.enter_context(nc.semaphore("s_cmp"))  # DVE rows done

    # --- SP stream: input DMAs (serial FIFO on the HWDGE ring) ---
        nc.sync.dma_start(out=wall_t.ap(), in_=wall[:]).then_inc(s_m, 16)
        for r in range(RPC):
            if r >= BUFS:  # WAR: slot reused once its row is consumed
                nc.sync.wait_ge(s_cmp, r - BUFS + 1)
            slot = xbuf[r % BUFS].ap()
            if r == LAST:  # split the last row so DVE overlaps its DMA
                nc.sync.dma_start(out=slot[:, 0:H1], in_=xs[r][:, 0:H1]).then_inc(
                    s_x[r % BUFS], 16
                )
                nc.sync.dma_start(out=slot[:, H1:H2], in_=xs[r][:, H1:H2]).then_inc(
                    s_l1, 16
                )
                nc.sync.dma_start(out=slot[:, H2:G], in_=xs[r][:, H2:G]).then_inc(
                    s_l2, 16
                )
            else:
                nc.sync.dma_start(out=slot, in_=xs[r]).then_inc(s_x[r % BUFS], 16)
        nc.sync.wait_ge(s_cmp, RPC)
        nc.sync.dma_start(out=out[:], in_=accall.ap()).then_inc(s_m, 16)
        nc.sync.wait_ge(s_m, 32)

        # --- DVE stream: fused multiply+reduce per mask per row. The
        # race detector needs an explicit completion wait before scratch
        # reuse (write-ack is async); one s_cmp wait per row covers all
        # three scratches via same-engine program order. ---
        nc.vector.wait_ge(s_m, 16)
        for r in range(RPC):
            nc.vector.wait_ge(s_x[r % BUFS], 16 * (r // BUFS + 1))
            xt = xbuf[r % BUFS].ap()
            if r > 0:
                nc.vector.wait_ge(s_cmp, r)  # scrA/B/C free (row r-1 done)
            if r == LAST:
                nc.vector.scalar_tensor_tensor(
                    out=scrA.ap()[:, 0:H1], in0=xt[:, 0:H1], scalar=1.0,
                    in1=wa_t[:, 0:H1], op0=mult, op1=mult,
                    accum_out=accA[:, r : r + 1],
                )
                nc.vector.scalar_tensor_tensor(
                    out=scrB.ap(), in0=xt[:, 0:SB], scalar=1.0, in1=wb_t,
                    op0=mult, op1=mult,
                    accum_out=accB[:, r : r + 1],
                )
                nc.vector.wait_ge(s_l1, 16)
                nc.vector.scalar_tensor_tensor(
                    out=scrA.ap()[:, H1:H2], in0=xt[:, H1:H2], scalar=1.0,
                    in1=wa_t[:, H1:H2], op0=mult, op1=mult,
                    accum_out=accA[:, r + 1 : r + 2],
                )
                nc.vector.wait_ge(s_l2, 16)
                nc.vector.scalar_tensor_tensor(
                    out=scrA.ap()[:, H2:G], in0=xt[:, H2:G], scalar=1.0,
                    in1=wa_t[:, H2:G], op0=mult, op1=mult,
                    accum_out=accA[:, r + 2 : r + 3],
                )
                nc.vector.scalar_tensor_tensor(
                    out=scrC.ap(), in0=xt[:, G - SC : G], scalar=1.0,
                    in1=wc_t, op0=mult, op1=mult,
                    accum_out=accC[:, r : r + 1],
                ).then_inc(s_cmp, 1)
            else:
                nc.vector.scalar_tensor_tensor(
                    out=scrA.ap(), in0=xt, scalar=1.0, in1=wa_t,
                    op0=mult, op1=mult,
                    accum_out=accA[:, r : r + 1],
                )
                nc.vector.scalar_tensor_tensor(
                    out=scrB.ap(), in0=xt[:, 0:SB], scalar=1.0, in1=wb_t,
                    op0=mult, op1=mult,
                    accum_out=accB[:, r : r + 1],
                )
                nc.vector.scalar_tensor_tensor(
                    out=scrC.ap(), in0=xt[:, G - SC : G], scalar=1.0,
                    in1=wc_t, op0=mult, op1=mult,
                    accum_out=accC[:, r : r + 1],
                ).then_inc(s_cmp, 1)
    return nc


def _prep_masks(w: np.ndarray, idx: np.ndarray):
    """Fold w into block-aligned fp16 masks. Returns None if idx strays
    more than one block from uniform (never happens for near-uniform
    sorted idx; caller falls back to a CPU scatter)."""
    blk = np.arange(N, dtype=np.int64) // G
    d = np.asarray(idx, np.int64) - blk
    if not bool(np.all(np.abs(d) <= 1)):
        return None
    col = np.arange(N, dtype=np.int64) % G
    wmA = np.where(d == 0, w, 0.0).astype(np.float32).reshape(P, G)

    selB = d == -1
    SB = int(col[selB].max()) + 1 if selB.any() else 1
    SB = min(G, max(16, -(-SB // 16) * 16))
    wmB = np.zeros((P, SB), np.float32)
    wmB[blk[selB], col[selB]] = w[selB]

    selC = d == 1
    SC = G - int(col[selC].min()) if selC.any() else 1
    SC = min(G, max(16, -(-SC // 16) * 16))
    wmC = np.zeros((P, SC), np.float32)
    wmC[blk[selC], col[selC] - (G - SC)] = w[selC]
    return wmA, wmB, wmC, SB, SC


def _segment_reduce_device(x: np.ndarray, wmA, wmB, wmC, SB, SC) -> np.ndarray:
    global LAST_RESULT
    key = (SB, SC)
    nc = _nc_cache.get(key)
    if nc is None:
        nc = _build_bass(SB, SC)
        _nc_cache[key] = nc

    xr = x.reshape(B, P, G)
    wall = np.ascontiguousarray(np.concatenate([wmA, wmB, wmC], axis=1))
    in_maps = [
        {
            "xs": xr[c * RPC : (c + 1) * RPC],
            "wall": wall,
        }
        for c in range(NCORES)
    ]
    LAST_RESULT = run_bass_kernel_spmd(
        nc, in_maps, core_ids=list(range(NCORES)), trace=TRACE
    )
    results = LAST_RESULT.results

    WXT = np.zeros((P, B), np.float32)
    for c in range(NCORES):
        o = results[c]["out"]
        a = o[:, : RPC + 2]
        b = o[:, RPC + 2 : 2 * RPC + 2]
        cc = o[:, 2 * RPC + 2 :]
        cols = slice(c * RPC, (c + 1) * RPC)
        wxt = np.empty((P, RPC), np.float32)
        wxt[:, : RPC - 1] = a[:, : RPC - 1]
        wxt[:, RPC - 1] = a[:, RPC - 1] + a[:, RPC] + a[:, RPC + 1]  # 3 pieces
        wxt[: P - 1] += b[1:]  # block p's spill belongs to segment p-1
        wxt[1:] += cc[: P - 1]  # block p's spill belongs to segment p+1
        WXT[:, cols] = wxt
    return WXT.T


def _segment_reduce_cpu(x: np.ndarray, w: np.ndarray, idx: np.ndarray):
    WX = np.zeros((B, P), np.float32)
    np.add.at(WX.T, np.asarray(idx, np.int64), (x * w).T)
    return WX


def _bn(z, g, b):
    m = z.mean(axis=0)
    v = np.mean((z - m) ** 2, axis=0)
    return (z - m) / np.sqrt(v + EPS) * g + b


def kernel(**inputs) -> tuple:
    x = np.ascontiguousarray(np.asarray(inputs["x"], np.float32))
    w = np.asarray(inputs["w"], np.float32)
    idx = np.asarray(inputs["idx"])

    masks = _prep_masks(w, idx)
    if masks is not None:
        WX = _segment_reduce_device(x, *masks)
    else:
        WX = _segment_reduce_cpu(x, w, idx)

    # Tiny MLP head on host ([64, 128] scale).
    f = np.float32
    Z = _bn(
        np.maximum(WX, 0),
        np.asarray(inputs["bn0_g"], f),
        np.asarray(inputs["bn0_b"], f),
    )
    Z = Z * (1.0 / (1.0 + np.exp(-np.asarray(inputs["co_w"], f))))
    h = _bn(
        np.maximum(Z @ np.asarray(inputs["W1"], f) + np.asarray(inputs["b1"], f), 0),
        np.asarray(inputs["bn1_g"], f),
        np.asarray(inputs["bn1_b"], f),
    )
    h = _bn(
        np.maximum(h @ np.asarray(inputs["W2"], f) + np.asarray(inputs["b2"], f), 0),
        np.asarray(inputs["bn2_g"], f),
        np.asarray(inputs["bn2_b"], f),
    )
    logits = h @ np.asarray(inputs["Wo"], f) + np.asarray(inputs["bo"], f)
    logits = logits - logits.max(axis=-1, keepdims=True)
    e = np.exp(logits)
    y = e / e.sum(axis=-1, keepdims=True)
    return (y.astype(np.float32), Z.astype(np.float32))


# revision 16
# speedup vs baseline: 1.4545x; 1.0377x over previous
"""MethylSPWNet forward pass on 8 Trainium2 NeuronCores.

Heavy part: WX[b, p] = sum_{i: idx[i]==p} x[b, i] * w[i]  (segment reduce,
x is [64, 800000] f32, idx sorted, 128 segments of ~6250).

Strategy (batch-parallel, 8 rows of x per core):
  - Per batch row, view the 800000 features as [128 partitions, 6250]
    (partition p = contiguous feature block p). Since idx is sorted and
    segments are near-uniform, segment p lives almost entirely in block p,
    spilling only a few hundred features into the edges of blocks p+-1.
  - Host folds w into three masks: wmA (features whose segment == own
    block), wmB (segment == block-1, nonzero only in leading columns),
    wmC (segment == block+1, nonzero only in trailing columns).
  - Device: one DMA per row ([128, 6250], fully contiguous per partition)
    plus three fused DVE scalar_tensor_tensor ops (elementwise multiply +
    free-axis reduce in a single pass) -> per-row accumulators [128, 1].
  - Host combines the partition-shifted accumulators into WX, then runs
    the tiny BN/CancelOut/MLP/softmax tail ([64,128] scale) in numpy.

DMA traffic is x's 25.6 MB per core + 3.7 MB of masks -> memory roofline.
The last row is split in pieces so the tail DVE op overlaps its own DMA.
"""

import sys

import numpy as np

if "/opt/trn_rl_repo" not in sys.path:
    sys.path.insert(0, "/opt/trn_rl_repo")

from contextlib import ExitStack

import concourse.bass as bass
import concourse.mybir as mybir
from concourse.bass_utils import run_bass_kernel_spmd

B, N, P = 64, 800000, 128
G = N // P  # 6250 features per partition block
# Last row is DMA'd and reduced in three pieces so the final DVE op after
# the last byte lands is small: [0:H1), [H1:H2), [H2:G)
H1 = 3125
H2 = 5461
NCORES = 8
RPC = B // NCORES  # batch rows per core
EPS = 1e-5

TRACE = False  # test harness sets True to collect an NTFF profile
LAST_RESULT = None  # BassKernelResults of the most recent device run

_nc_cache: dict = {}


BUFS = 4  # x-row buffer slots


def _build_bass(SB: int, SC: int) -> bass.Bass:
    """Raw bass (manual semaphores): SP issues DMAs, DVE does one fused
    multiply+reduce per mask per row. Tile's auto-generated kernel-tail
    drain trips a walrus 'too many sync waits' limit in this container,
    so the pipeline is hand-synced.

    Semaphore discipline: a wait for the total of a *closed set* of DMAs
    is safe (all must finish to reach the total); a partial count over
    concurrent DMAs is not, because the 16 per-SDMA-engine increments of
    different DMAs interleave. Hence one sem per x-row slot use.
    """
    nc = bass.Bass()
    f32 = mybir.dt.float32
    i16 = mybir.dt.int16
    LAST = RPC - 1
    xs = nc.dram_tensor("xs", [RPC, P, G], f32, kind="ExternalInput")
    # masks are int16 fixed-point (global scale folded out on host): exact
    # int->fp32 conversion in the DVE ALU, half the HBM bytes of f32
    wall = nc.dram_tensor("wall", [P, G + SB + SC], i16, kind="ExternalInput")
    # out cols: [0:RPC+2) = A (last row split in three pieces),
    # [RPC+2 : 2*RPC+2) = B, [2*RPC+2 : 3*RPC+2) = C
    out = nc.dram_tensor("out", [P, 3 * RPC + 2], f32, kind="ExternalOutput")

    mult = mybir.AluOpType.mult

    with ExitStack() as ctx:
        ctx.enter_context(nc.Block())
        wall_t = ctx.enter_context(nc.sbuf_tensor("wall_t", [P, G + SB + SC], i16))
        wa_t = wall_t.ap()[:, 0:G]
        wb_t = wall_t.ap()[:, G : G + SB]
        wc_t = wall_t.ap()[:, G + SB : G + SB + SC]
        accall = ctx.enter_context(nc.sbuf_tensor("accall", [P, 3 * RPC + 2], f32))
        accA = accall.ap()[:, 0 : RPC + 2]
        accB = accall.ap()[:, RPC + 2 : 2 * RPC + 2]
        accC = accall.ap()[:, 2 * RPC + 2 : 3 * RPC + 2]
        scrA = ctx.enter_context(nc.sbuf_tensor("scrA", [P, G], f32))
        scrB = ctx.enter_context(nc.sbuf_tensor("scrB", [P, SB], f32))
        scrC = ctx.enter_context(nc.sbuf_tensor("scrC", [P, SC], f32))
        xbuf = [
            ctx.enter_context(nc.sbuf_tensor(f"xb{i}", [P, G], f32))
            for i in range(BUFS)
        ]
        s_m = ctx.enter_context(nc.semaphore("s_m"))  # packed masks + out
        s_x = [ctx.enter_context(nc.semaphore(f"s_x{i}")) for i in range(BUFS)]
        s_l1 = ctx.enter_context(nc.semaphore("s_l1"))  # last row piece 2
        s_l2 = ctx.enter_context(nc.semaphore("s_l2"))  # last row piece 3
        s_cmp = ctx.enter_context(nc.semaphore("s_cmp"))  # DVE rows done

    # --- SP stream: input DMAs (serial FIFO on the HWDGE ring) ---
        nc.sync.dma_start(out=wall_t.ap(), in_=wall[:]).then_inc(s_m, 16)
        for r in range(RPC):
            if r >= BUFS:  # WAR: slot reused once its row is consumed
                nc.sync.wait_ge(s_cmp, r - BUFS + 1)
            slot = xbuf[r % BUFS].ap()
            if r == LAST:  # split the last row so DVE overlaps its DMA
                nc.sync.dma_start(out=slot[:, 0:H1], in_=xs[r][:, 0:H1]).then_inc(
                    s_x[r % BUFS], 16
                )
                nc.sync.dma_start(out=slot[:, H1:H2], in_=xs[r][:, H1:H2]).then_inc(
                    s_l1, 16
                )
                nc.sync.dma_start(out=slot[:, H2:G], in_=xs[r][:, H2:G]).then_inc(
                    s_l2, 16
                )
            else:
                nc.sync.dma_start(out=slot, in_=xs[r]).then_inc(s_x[r % BUFS], 16)
        nc.sync.wait_ge(s_cmp, RPC)
        nc.sync.dma_start(out=out[:], in_=accall.ap()).then_inc(s_m, 16)
        nc.sync.wait_ge(s_m, 32)

        # --- DVE stream: fused multiply+reduce per mask per row. The
        # race detector needs an explicit completion wait before scratch
        # reuse (write-ack is async); one s_cmp wait per row covers all
        # three scratches via same-engine program order. ---
        nc.vector.wait_ge(s_m, 16)
        for r in range(RPC):
            nc.vector.wait_ge(s_x[r % BUFS], 16 * (r // BUFS + 1))
            xt = xbuf[r % BUFS].ap()
            if r > 0:
                nc.vector.wait_ge(s_cmp, r)  # scrA/B/C free (row r-1 done)
            if r == LAST:
                nc.vector.scalar_tensor_tensor(
                    out=scrA.ap()[:, 0:H1], in0=xt[:, 0:H1], scalar=1.0,
                    in1=wa_t[:, 0:H1], op0=mult, op1=mult,
                    accum_out=accA[:, r : r + 1],
                )
                nc.vector.scalar_tensor_tensor(
                    out=scrB.ap(), in0=xt[:, 0:SB], scalar=1.0, in1=wb_t,
                    op0=mult, op1=mult,
                    accum_out=accB[:, r : r + 1],
                )
                nc.vector.wait_ge(s_l1, 16)
                nc.vector.scalar_tensor_tensor(
                    out=scrA.ap()[:, H1:H2], in0=xt[:, H1:H2], scalar=1.0,
                    in1=wa_t[:, H1:H2], op0=mult, op1=mult,
                    accum_out=accA[:, r + 1 : r + 2],
                )
                nc.vector.wait_ge(s_l2, 16)
                nc.vector.scalar_tensor_tensor(
                    out=scrA.ap()[:, H2:G], in0=xt[:, H2:G], scalar=1.0,
                    in1=wa_t[:, H2:G], op0=mult, op1=mult,
                    accum_out=accA[:, r + 2 : r + 3],
                )
                nc.vector.scalar_tensor_tensor(
                    out=scrC.ap(), in0=xt[:, G - SC : G], scalar=1.0,
                    in1=wc_t, op0=mult, op1=mult,
                    accum_out=accC[:, r : r + 1],
                ).then_inc(s_cmp, 1)
            else:
                nc.vector.scalar_tensor_tensor(
                    out=scrA.ap(), in0=xt, scalar=1.0, in1=wa_t,
                    op0=mult, op1=mult,
                    accum_out=accA[:, r : r + 1],
                )
                nc.vector.scalar_tensor_tensor(
                    out=scrB.ap(), in0=xt[:, 0:SB], scalar=1.0, in1=wb_t,
                    op0=mult, op1=mult,
                    accum_out=accB[:, r : r + 1],
                )
                nc.vector.scalar_tensor_tensor(
                    out=scrC.ap(), in0=xt[:, G - SC : G], scalar=1.0,
                    in1=wc_t, op0=mult, op1=mult,
                    accum_out=accC[:, r : r + 1],
                ).then_inc(s_cmp, 1)
    return nc


def _prep_masks(w: np.ndarray, idx: np.ndarray):
    """Fold w into block-aligned fp16 masks. Returns None if idx strays
    more than one block from uniform (never happens for near-uniform
    sorted idx; caller falls back to a CPU scatter)."""
    blk = np.arange(N, dtype=np.int64) // G
    d = np.asarray(idx, np.int64) - blk
    if not bool(np.all(np.abs(d) <= 1)):
        return None
    col = np.arange(N, dtype=np.int64) % G
    wmA = np.where(d == 0, w, 0.0).astype(np.float32).reshape(P, G)

    selB = d == -1
    SB = int(col[selB].max()) + 1 if selB.any() else 1
    SB = min(G, max(16, -(-SB // 16) * 16))
    wmB = np.zeros((P, SB), np.float32)
    wmB[blk[selB], col[selB]] = w[selB]

    selC = d == 1
    SC = G - int(col[selC].min()) if selC.any() else 1
    SC = min(G, max(16, -(-SC // 16) * 16))
    wmC = np.zeros((P, SC), np.float32)
    wmC[blk[selC], col[selC] - (G - SC)] = w[selC]

    S = np.float32(32000.0 / max(np.abs(w).max(), 1e-30))
    q = lambda m: np.clip(np.rint(m * S), -32767, 32767).astype(np.int16)
    return q(wmA), q(wmB), q(wmC), SB, SC, np.float32(1.0 / S)


def _segment_reduce_device(x: np.ndarray, wmA, wmB, wmC, SB, SC, invS) -> np.ndarray:
    global LAST_RESULT
    key = (SB, SC)
    nc = _nc_cache.get(key)
    if nc is None:
        nc = _build_bass(SB, SC)
        _nc_cache[key] = nc

    xr = x.reshape(B, P, G)
    wall = np.ascontiguousarray(np.concatenate([wmA, wmB, wmC], axis=1))
    in_maps = [
        {
            "xs": xr[c * RPC : (c + 1) * RPC],
            "wall": wall,
        }
        for c in range(NCORES)
    ]
    LAST_RESULT = run_bass_kernel_spmd(
        nc, in_maps, core_ids=list(range(NCORES)), trace=TRACE
    )
    results = LAST_RESULT.results

    WXT = np.zeros((P, B), np.float32)
    for c in range(NCORES):
        o = results[c]["out"] * invS
        a = o[:, : RPC + 2]
        b = o[:, RPC + 2 : 2 * RPC + 2]
        cc = o[:, 2 * RPC + 2 :]
        cols = slice(c * RPC, (c + 1) * RPC)
        wxt = np.empty((P, RPC), np.float32)
        wxt[:, : RPC - 1] = a[:, : RPC - 1]
        wxt[:, RPC - 1] = a[:, RPC - 1] + a[:, RPC] + a[:, RPC + 1]  # 3 pieces
        wxt[: P - 1] += b[1:]  # block p's spill belongs to segment p-1
        wxt[1:] += cc[: P - 1]  # block p's spill belongs to segment p+1
        WXT[:, cols] = wxt
    return WXT.T


def _segment_reduce_cpu(x: np.ndarray, w: np.ndarray, idx: np.ndarray):
    WX = np.zeros((B, P), np.float32)
    np.add.at(WX.T, np.asarray(idx, np.int64), (x * w).T)
    return WX


def _bn(z, g, b):
    m = z.mean(axis=0)
    v = np.mean((z - m) ** 2, axis=0)
    return (z - m) / np.sqrt(v + EPS) * g + b


def kernel(**inputs) -> tuple:
    x = np.ascontiguousarray(np.asarray(inputs["x"], np.float32))
    w = np.asarray(inputs["w"], np.float32)
    idx = np.asarray(inputs["idx"])

    masks = _prep_masks(w, idx)
    if masks is not None:
        WX = _segment_reduce_device(x, *masks)
    else:
        WX = _segment_reduce_cpu(x, w, idx)

    # Tiny MLP head on host ([64, 128] scale).
    f = np.float32
    Z = _bn(
        np.maximum(WX, 0),
        np.asarray(inputs["bn0_g"], f),
        np.asarray(inputs["bn0_b"], f),
    )
    Z = Z * (1.0 / (1.0 + np.exp(-np.asarray(inputs["co_w"], f))))
    h = _bn(
        np.maximum(Z @ np.asarray(inputs["W1"], f) + np.asarray(inputs["b1"], f), 0),
        np.asarray(inputs["bn1_g"], f),
        np.asarray(inputs["bn1_b"], f),
    )
    h = _bn(
        np.maximum(h @ np.asarray(inputs["W2"], f) + np.asarray(inputs["b2"], f), 0),
        np.asarray(inputs["bn2_g"], f),
        np.asarray(inputs["bn2_b"], f),
    )
    logits = h @ np.asarray(inputs["Wo"], f) + np.asarray(inputs["bo"], f)
    logits = logits - logits.max(axis=-1, keepdims=True)
    e = np.exp(logits)
    y = e / e.sum(axis=-1, keepdims=True)
    return (y.astype(np.float32), Z.astype(np.float32))
